# revision 2
# baseline (speedup 1.0000x reference)
"""Trainium2 Bass kernel for the tanh-RNN problem.

Reference computation (per batch row b):
    xproj = input @ wi + brec                 # [B, T, H]
    h_{t+1} = 0.5*h_t + 0.5*tanh(h_t @ wrec.T + xproj_t)
    output  = hs @ wo                         # [B, T, O]

Strategy (8 cores, data-parallel over batch, B_local = 8):
  - State kept H-major ("gT" layout [H, B_local]) as g = 2*h, so the blend is
    g' = 0.5*g + tanh(.) -- one fused DVE scalar_tensor_tensor op. The /2 is
    folded into wrec and wo on the host.
  - Per step: 16 accumulating matmuls (4 m-tiles x 4 k-tiles) with the fp16
    wrec blocks as the stationary operand (FWL) and the g k-tiles [128, 8] as
    the moving operand, accumulating onto a PSUM bank that was preloaded with
    xproj for a 64-step chunk (one [65,128]x[65,512] matmul per m per chunk;
    brec enters via an augmented ones-row of x).
  - tanh on ACT straight out of PSUM; blend on DVE writes the fp16 history
    buffer HT which doubles as the next step's matmul input and the output
    projection input.
  - Output projection at the tail: outT[64, T*8] = (wo/2).T @ g, 4 k-matmuls
    per 512-column chunk, evicted PSUM->SBUF->HBM.
"""

import os

import numpy as np

import concourse.bacc as bacc
import concourse.mybir as mybir
from concourse.tile import TileContext
from concourse import bass_utils

F16 = mybir.dt.float16
F32 = mybir.dt.float32

B, T_FULL, I, H, O = 64, 1024, 64, 512, 64
NCORES = 8
BL = B // NCORES          # batch rows per core
KT = H // 128             # 4 tiles over H
CHUNK = 64                # steps per PSUM bank (64*8 = 512 fp32 cols)

# wavefront order of the 16 (m, k) matmuls within a step, crafted so the
# h_k produced latest in step t is consumed latest in step t+1
MM_ORDER = [
    (0, 0), (1, 0), (0, 1), (1, 1), (0, 2), (1, 2), (2, 0), (0, 3),
    (1, 3), (2, 1), (3, 0), (2, 2), (2, 3), (3, 1), (3, 2), (3, 3),
]
# slot index of each m-group's last matmul (where its ACT+DVE go)
_LAST_SLOT = {m: max(i for i, (mm, _) in enumerate(MM_ORDER) if mm == m) for m in range(KT)}


def build(t_steps: int = T_FULL):
    assert t_steps % CHUNK == 0
    nchunk = t_steps // CHUNK
    cols = t_steps * BL            # free-dim columns of xT/outT
    htw = (t_steps + 1) * BL       # per-m width of the history buffer

    nc = bacc.Bacc("TRN2", target_bir_lowering=False, debug=False)

    d_wT = nc.dram_tensor("wT", [KT, 128, H], F16, kind="ExternalInput")
    d_wi = nc.dram_tensor("wiA", [I + 1, H], F16, kind="ExternalInput")
    d_wo = nc.dram_tensor("woT", [KT, 128, O], F16, kind="ExternalInput")
    d_g0 = nc.dram_tensor("g0", [KT, 128, BL], F16, kind="ExternalInput")
    d_xT = nc.dram_tensor("xT", [I + 1, cols], F16, kind="ExternalInput")
    d_out = nc.dram_tensor("outT", [O, cols], F32, kind="ExternalOutput")

    with TileContext(nc) as tc:
        with (
            tc.tile_pool(name="wpool", bufs=1) as wpool,
            tc.tile_pool(name="ht", bufs=1) as htpool,
            tc.tile_pool(name="r", bufs=8) as rpool,
            tc.tile_pool(name="osb", bufs=4) as opool,
            tc.tile_pool(name="px", bufs=8, space="PSUM") as px,
        ):
            wT = [wpool.tile([128, H], F16, tag=f"wT{k}", name=f"wT{k}") for k in range(KT)]
            for k in range(KT):
                nc.sync.dma_start(wT[k][:], d_wT[k])
            wi = wpool.tile([I + 1, H], F16, tag="wi")
            nc.sync.dma_start(wi[:], d_wi[:])
            wo = [wpool.tile([128, O], F16, tag=f"wo{k}", name=f"wo{k}") for k in range(KT)]
            for k in range(KT):
                nc.sync.dma_start(wo[k][:], d_wo[k])
            xT = wpool.tile([I + 1, cols], F16, tag="xT")
            nc.sync.dma_start(xT[:], d_xT[:])

            # history buffer: m-major blocks of width htw; col t*BL.. holds g_t
            HT = htpool.tile([128, KT * htw], F16, tag="HT")
            for m in range(KT):
                nc.sync.dma_start(HT[:, m * htw : m * htw + BL], d_g0[m])

            def refill(c):
                """xproj chunk c -> 4 fresh psum tiles (preloaded a-banks)."""
                tiles = []
                for m in range(KT):
                    pt = px.tile([128, CHUNK * BL], F32, tag="px", name=f"px{c}_{m}")
                    nc.tensor.matmul(
                        pt[:],
                        lhsT=wi[:, m * 128 : (m + 1) * 128],
                        rhs=xT[:, c * CHUNK * BL : (c + 1) * CHUNK * BL],
                        start=True,
                        stop=False,
                        skip_group_check=True,
                    )
                    tiles.append(pt)
                return tiles

            cur = refill(0)
            nxt = None
            for t in range(t_steps):
                c, tt = divmod(t, CHUNK)
                if tt == 1 and c + 1 < nchunk:
                    nxt = refill(c + 1)
                acts = {}
                for slot, (m, k) in enumerate(MM_ORDER):
                    nc.tensor.matmul(
                        cur[m][:, tt * BL : (tt + 1) * BL],
                        lhsT=wT[k][:, m * 128 : (m + 1) * 128],
                        rhs=HT[:, k * htw + t * BL : k * htw + (t + 1) * BL],
                        start=False,
                        stop=(tt == CHUNK - 1 and slot == _LAST_SLOT[m]),
                        skip_group_check=True,
                    )
                    if slot == _LAST_SLOT[m]:
                        r = rpool.tile([128, BL], F32, tag="r", name=f"r{t}_{m}")
                        nc.scalar.activation(
                            r[:],
                            cur[m][:, tt * BL : (tt + 1) * BL],
                            mybir.ActivationFunctionType.Tanh,
                        )
                        nc.vector.scalar_tensor_tensor(
                            HT[:, m * htw + (t + 1) * BL : m * htw + (t + 2) * BL],
                            in0=HT[:, m * htw + t * BL : m * htw + (t + 1) * BL],
                            scalar=0.5,
                            in1=r[:],
                            op0=mybir.AluOpType.mult,
                            op1=mybir.AluOpType.add,
                        )
                        acts[m] = True
                if tt == CHUNK - 1:
                    cur = nxt
                    nxt = None

            # ---- output projection tail: outT = (wo/2).T @ g ----
            for c in range(nchunk):
                po = px.tile([O, CHUNK * BL], F32, tag="px", name=f"po{c}")
                for k in range(KT):
                    nc.tensor.matmul(
                        po[:],
                        lhsT=wo[k][:],
                        rhs=HT[:, k * htw + BL + c * CHUNK * BL : k * htw + BL + (c + 1) * CHUNK * BL],
                        start=(k == 0),
                        stop=(k == KT - 1),
                    )
                ot = opool.tile([O, CHUNK * BL], F32, tag="osb", name=f"ot{c}")
                nc.vector.tensor_copy(ot[:], po[:])
                nc.sync.dma_start(d_out[:, c * CHUNK * BL : (c + 1) * CHUNK * BL], ot[:])

    nc.compile()
    return nc


_CACHE = {}


def _get_nc(t_steps):
    if t_steps not in _CACHE:
        _CACHE[t_steps] = build(t_steps)
    return _CACHE[t_steps]


def prep_inputs(input, wi, wrec, wo, brec, h0, t_steps):
    """Host-side shard + layout prep. Returns list of 8 in_maps."""
    input = np.asarray(input, dtype=np.float32)
    wi = np.asarray(wi, dtype=np.float32)
    wrec = np.asarray(wrec, dtype=np.float32)
    wo = np.asarray(wo, dtype=np.float32)
    brec = np.asarray(brec, dtype=np.float32)
    h0 = np.asarray(h0, dtype=np.float32)

    wT = np.ascontiguousarray((wrec.T / 2.0).astype(np.float16).reshape(KT, 128, H))
    wiA = np.concatenate([wi, brec[None, :]], axis=0).astype(np.float16)  # [I+1, H]
    woT = np.ascontiguousarray((wo / 2.0).astype(np.float16).reshape(KT, 128, O))
    g0 = np.broadcast_to((2.0 * h0).astype(np.float16)[:, None], (H, BL))
    g0 = np.ascontiguousarray(g0.reshape(KT, 128, BL))

    in_maps = []
    for c in range(NCORES):
        xc = input[c * BL : (c + 1) * BL, :t_steps, :]          # [BL, t, I]
        xT = np.ascontiguousarray(np.transpose(xc, (2, 1, 0)).reshape(I, t_steps * BL))
        xA = np.concatenate(
            [xT, np.ones((1, t_steps * BL), np.float32)], axis=0
        ).astype(np.float16)                                     # [I+1, t*BL]
        in_maps.append({"wT": wT, "wiA": wiA, "woT": woT, "g0": g0, "xT": xA})
    return in_maps


def run_sharded(inputs, t_steps=T_FULL, trace=False):
    nc = _get_nc(t_steps)
    in_maps = prep_inputs(**inputs, t_steps=t_steps)
    res = bass_utils.run_bass_kernel_spmd(
        nc, in_maps, core_ids=list(range(NCORES)), trace=trace
    )
    outs = []
    for c in range(NCORES):
        oT = res.results[c]["outT"]                              # [O, t*BL]
        outs.append(np.transpose(oT.reshape(O, t_steps, BL), (2, 1, 0)))
    return np.concatenate(outs, axis=0).astype(np.float32), res


def kernel(input, wi, wrec, wo, brec, h0):
    out, _ = run_sharded(
        dict(input=input, wi=wi, wrec=wrec, wo=wo, brec=brec, h0=h0),
        t_steps=T_FULL,
        trace=False,
    )
    return out
